# revision 30
# baseline (speedup 1.0000x reference)
"""Block-circulant matmul kernel for 8 Trainium2 NeuronCores.

Reference op (per token row x of shape (4096,)):
    y = (x*d) @ M + bias,  M[(j,m),(i,n)] = W[i,j,(m-n)%256]  (circulant blocks)

Implementation (default "fft"): real-DFT factorization executed in three
matmul stages per core, data-parallel over the batch (1024 tokens/core):
  stage1: per input block j, project onto the 256-col real DFT basis (fp32r)
  stage2: per frequency-group G (4 pair-slots), one 128x128 block-diag mix (bf16)
  stage3: per output block i, inverse real DFT basis + bias (bf16)
Between stages, two SBUF->SBUF partition-shuffle DMA passes regroup the
data (frequency-major <-> block-major).  ~7.6x fewer FLOPs than dense.

"dense" fallback: y^T = M^T x^T as a plain fp32r matmul.

Self-contained: shapes hardcoded; no sibling imports.
"""
import os
import sys

for _p in ("/root/.axon_site", "/root/.axon_site/_ro/trn_rl_repo", "/root/.axon_site/_ro/pypackages"):
    if _p not in sys.path:
        sys.path.append(_p)

import numpy as np
import ml_dtypes

import concourse.bass as bass
import concourse.tile as tile
from concourse import bacc, mybir
from concourse import bass_utils

N_CORES = 8
B = 8192
D = 4096
BS = 256
K = 16             # blocks per side
NSLOT = BS // 2    # 128 frequency pair-slots
NT = B // N_CORES  # tokens per core (1024)
TC = 512           # token chunk
NCH = NT // TC     # chunks (2)

F32 = mybir.dt.float32
F32R = mybir.dt.float32r
BF16 = mybir.dt.bfloat16
BF16_NP = ml_dtypes.bfloat16

LAST_EXEC_NS = None
_CACHE = {}


# ---------------------------------------------------------------- host math

def _canonical_mats(W):
    m = np.arange(BS)
    T = np.zeros((BS, BS), np.float64)
    T[:, 0] = 1.0
    T[:, 1] = (-1.0) ** m
    for f in range(1, NSLOT):
        T[:, 2 * f] = np.cos(2 * np.pi * f * m / BS)
        T[:, 2 * f + 1] = np.sin(2 * np.pi * f * m / BS)

    Wf = np.fft.fft(W.astype(np.float64), axis=-1)
    p = Wf.real
    q = -Wf.imag

    jj = np.arange(K)
    M_slot = np.zeros((NSLOT, 2 * K, 2 * K), np.float64)
    for f in range(1, NSLOT):
        pf, qf = p[:, :, f], q[:, :, f]          # [i, j]
        M_slot[f][np.ix_(2 * jj, 2 * jj)] = pf.T
        M_slot[f][np.ix_(2 * jj + 1, 2 * jj)] = qf.T
        M_slot[f][np.ix_(2 * jj, 2 * jj + 1)] = qf.T
        M_slot[f][np.ix_(2 * jj + 1, 2 * jj + 1)] = -pf.T
    M_slot[0][np.ix_(2 * jj, 2 * jj)] = p[:, :, 0].T
    M_slot[0][np.ix_(2 * jj + 1, 2 * jj + 1)] = p[:, :, NSLOT].T

    n = np.arange(BS)
    R = np.zeros((BS, BS), np.float64)
    R[0, :] = 1.0 / BS
    R[1, :] = ((-1.0) ** n) / BS
    for f in range(1, NSLOT):
        R[2 * f, :] = 2.0 / BS * np.cos(2 * np.pi * f * n / BS)
        R[2 * f + 1, :] = -2.0 / BS * np.sin(2 * np.pi * f * n / BS)
    return T, M_slot, R


def _fft_host_mats(W, bias):
    T, M_slot, R = _canonical_mats(W)
    p_idx = np.arange(128)

    # tb_dram (128, 4*128): [p, (mt*2+pb)*128+col] = T[mt*128+p, colmap(pb,col)]
    tb = np.zeros((128, 512), np.float32)
    for pb in range(2):
        slot = 64 * pb + 4 * (p_idx // 8) + (p_idx % 8) // 2
        c = p_idx % 2
        cols = 2 * slot + c                       # canonical comp per device col
        for mt in range(2):
            tb[:, (mt * 2 + pb) * 128:(mt * 2 + pb + 1) * 128] = \
                T[mt * 128:(mt + 1) * 128, :][:, cols]

    # mix_dram (128, 32*128) bf16: [row, G*128+col]
    mix = np.zeros((128, 32 * 128), np.float64)
    kk = np.arange(K)
    for G in range(32):
        MG = np.zeros((128, 128), np.float64)
        for r in range(4):
            blk = M_slot[4 * G + r]
            for c in range(2):
                for cp in range(2):
                    MG[np.ix_(16 * (2 * r + c) + kk, 16 * (2 * r + cp) + kk)] = \
                        blk[np.ix_(2 * kk + c, 2 * kk + cp)]
        mix[:, G * 128:(G + 1) * 128] = MG

    # r_dram (128, 4*128): [p, (kt*2+nb)*128+col] = R[rowmap(kt,p), nb*128+col]
    rd = np.zeros((128, 512), np.float64)
    for kt in range(2):
        gl = p_idx // 8
        q = (p_idx % 8) // 2
        c = p_idx % 2
        rows = 2 * (64 * kt + 4 * gl + q) + c
        for nb in range(2):
            rd[:, (kt * 2 + nb) * 128:(kt * 2 + nb + 1) * 128] = \
                R[rows, :][:, nb * 128:(nb + 1) * 128]

    # beta: per output block i solve R^T beta_i = bias_i; fold into stage-2
    # layout (128, 32) f32: [16*qc + i, G] = beta_i[2*(4G+q)+c], qc = 2q+c
    beta = np.zeros((128, 32), np.float64)
    RTinv = np.linalg.inv(R.T)
    for i in range(K):
        bi = RTinv @ bias[i * BS:(i + 1) * BS].astype(np.float64)
        for G in range(32):
            for q in range(4):
                for c in range(2):
                    qc = 2 * q + c
                    beta[16 * qc + i, G] = bi[2 * (4 * G + q) + c]
    return (tb.astype(BF16_NP),
            mix.astype(BF16_NP),
            rd.astype(BF16_NP),
            beta.astype(np.float32))


# ---------------------------------------------------------------- v2 host math

def _v2_host_mats(W):
    """Constants for the v2 kernel.

    Device component index c = g*8 + s for group g in [0,32), s in [0,8);
    slot = 4g + s//2, canonical column q = 2*slot + (s%2), matching
    _canonical_mats' T/R column/row order (0=DC, 1=Nyquist, 2f/2f+1=cos/sin).
    """
    T, M_slot, R = _canonical_mats(W)
    cdev = np.arange(256)
    g_of = cdev // 8
    s_of = cdev % 8
    q_of = 2 * (4 * g_of + s_of // 2) + (s_of % 2)

    # t_dev (128, 512): [p, (h*2+pb)*128 + ci] = T[h*128+p, q(pb*128+ci)]
    t = np.zeros((128, 512))
    for h in range(2):
        for pb in range(2):
            cols = q_of[pb * 128:(pb + 1) * 128]
            t[:, (h * 2 + pb) * 128:(h * 2 + pb + 1) * 128] = \
                T[h * 128:(h + 1) * 128, :][:, cols]

    # mix_dev (128, 32*128): row (j//8)*64 + s*8 + (j%8)  (u2's partition
    # layout, implied by the plain-AP shuffle DMA), col g*128 + i*8 + s2;
    # value (s//2 == s2//2) * M_slot[4g + s//2][2j + s%2, 2i + s2%2]
    mix = np.zeros((128, 32 * 128))
    jj = np.arange(K)
    prow = (jj[:, None] // 8) * 64 + np.arange(8)[None, :] * 8 + (jj[:, None] % 8)
    for g in range(32):
        for r in range(4):
            blk = M_slot[4 * g + r]
            for cb1 in range(2):
                for cb2 in range(2):
                    mix[np.ix_(prow[:, 2 * r + cb1],
                               g * 128 + 8 * jj + 2 * r + cb2)] = \
                        blk[np.ix_(2 * jj + cb1, 2 * jj + cb2)]

    # r_dev (128, 512): [cc, kt*256 + n] = R[q(kt*128+cc), n]
    rd = np.zeros((128, 512))
    for kt in range(2):
        rd[:, kt * 256:(kt + 1) * 256] = R[q_of[kt * 128:(kt + 1) * 128], :]

    ident = np.eye(128)
    return (t.astype(BF16_NP), mix.astype(BF16_NP), rd.astype(BF16_NP),
            ident.astype(BF16_NP))


# ---------------------------------------------------------------- v2 kernel

TCH = 128            # token chunk for stages 2/3 (transpose granularity)
NCH2 = NT // TCH     # 8 chunks


def _build_v2_nc():
    nc = bacc.Bacc("TRN2", target_bir_lowering=False, debug=False)
    # x_dev: row j*128+p, col h*NT + t  (4KB contiguous lines)
    xT = nc.dram_tensor("xT", [K * 128, 2 * NT], BF16, kind="ExternalInput").ap()
    t_d = nc.dram_tensor("tmat", [128, 512], BF16, kind="ExternalInput").ap()
    mix_d = nc.dram_tensor("mix", [128, 32 * 128], BF16, kind="ExternalInput").ap()
    r_d = nc.dram_tensor("rmat", [128, 512], BF16, kind="ExternalInput").ap()
    id_d = nc.dram_tensor("ident", [128, 128], BF16, kind="ExternalInput").ap()
    # y_dev: row tok (chunk*128+p), col feature  (8KB contiguous lines)
    yD = nc.dram_tensor("yD", [NT, D], BF16, kind="ExternalOutput").ap()

    ec = [0]
    n_evac = int(os.environ.get("KERNEL_NEVAC", "2"))

    def evac(dst, src):
        m = ec[0] % n_evac
        if m == 0:
            nc.vector.tensor_copy(dst, src)
        elif m == 1:
            nc.scalar.copy(dst, src)
        else:
            nc.gpsimd.tensor_copy(dst, src)
        ec[0] += 1

    with tile.TileContext(nc) as tc:
        with (
            tc.tile_pool(name="consts", bufs=1) as consts,
            tc.tile_pool(name="xpool", bufs=2) as xpool,
            tc.tile_pool(name="upool", bufs=1) as upool,
            tc.tile_pool(name="u2pool", bufs=1) as u2pool,
            tc.tile_pool(name="wpool", bufs=6) as wpool,
            tc.tile_pool(name="ypool", bufs=2) as ypool,
            tc.tile_pool(name="ps1p", bufs=2, space="PSUM") as ps1p,
            tc.tile_pool(name="ps2p", bufs=2, space="PSUM") as ps2p,
            tc.tile_pool(name="pstp", bufs=2, space="PSUM") as pstp,
            tc.tile_pool(name="ps3p", bufs=2, space="PSUM") as ps3p,
        ):
            # first stage-1 operands load before the big constants so the PE
            # can start as early as possible (t first: needed by the first mm)
            t_sb = consts.tile([128, 512], BF16)
            nc.sync.dma_start(t_sb[:], t_d[:])
            id_sb = consts.tile([128, 128], BF16)
            nc.scalar.dma_start(id_sb[:], id_d[:])
            mix_sb = consts.tile([128, 32 * 128], BF16)
            nc.scalar.dma_start(mix_sb[:], mix_d[:])
            r_sb = consts.tile([128, 512], BF16)
            nc.scalar.dma_start(r_sb[:], r_d[:])

            # ---- stage 1 (j-outer, x streamed): u[pb][c, j*NT+t] ----------
            u_sb = [upool.tile([128, K * NT], BF16, tag=f"u{pb}",
                               name=f"u{pb}") for pb in range(2)]
            # shuffle-1 output, split by queue ownership so no tensor is
            # written from two DMA queues: g<NSW -> SWDGE, then sync, scalar.
            NSW, NSY = 18, 7
            NSC = 32 - NSW - NSY
            u2a = u2pool.tile([128, NSW * NT], BF16, tag="u2a", name="u2a")
            u2b = u2pool.tile([128, NSY * NT], BF16, tag="u2b", name="u2b")
            u2c = u2pool.tile([128, NSC * NT], BF16, tag="u2c", name="u2c")

            def u2_ap(g, c0, c1):
                if g < NSW:
                    return u2a[:, g * NT + c0:g * NT + c1]
                if g < NSW + NSY:
                    gg = g - NSW
                    return u2b[:, gg * NT + c0:gg * NT + c1]
                gg = g - NSW - NSY
                return u2c[:, gg * NT + c0:gg * NT + c1]

            def sh1(jh):
                # u2_g[jh*64 + s*8 + jl, t] = u[pb][gl*8+s, (jh*8+jl)*NT+t].
                # Plain 2D APs on both sides (partition map implied by the
                # DMA balancer); full-NT runs (2KB packets). SWDGE only:
                # HWDGE triggers block the sync/scalar streams and starve
                # x-loads/evacs (measured), gpsimd has nothing else to do.
                for g in range(32):
                    pb, gl = g // 16, g % 16
                    dst = u2_ap(g, 0, NT)[jh * 64:(jh + 1) * 64, :]
                    src = u_sb[pb][8 * gl:8 * gl + 8,
                                   jh * 8 * NT:(jh + 1) * 8 * NT]
                    nc.gpsimd.dma_start(dst, src)

            for j in range(K):
                x_t = xpool.tile([128, 2 * NT], BF16, tag="x")
                if j % 2 == 0:
                    nc.sync.dma_start(x_t[:], xT[j * 128:(j + 1) * 128, :])
                else:
                    nc.scalar.dma_start(x_t[:], xT[j * 128:(j + 1) * 128, :])
                for pb in range(2):
                    for th in range(2):
                        ps1 = ps1p.tile([128, 512], F32, tag="ps1")
                        for h in range(2):
                            nc.tensor.matmul(
                                ps1[:],
                                t_sb[:, (h * 2 + pb) * 128:(h * 2 + pb + 1) * 128],
                                x_t[:, h * NT + th * 512:h * NT + (th + 1) * 512],
                                start=(h == 0), stop=(h == 1),
                            )
                        evac(u_sb[pb][:, j * NT + th * 512:j * NT + (th + 1) * 512],
                             ps1[:])
                if j == K // 2 - 1:
                    sh1(0)
            sh1(1)

            # ---- stages 2/3 per 128-token chunk, software-pipelined --------
            # w is i-major (col = i*256 + g*8 + s2); stage-2 packs 4 groups
            # per PSUM bank and evacuates with one 4D-scatter copy. T2 reads
            # contiguous ik blocks and writes v back in place (disjoint).
            w_ts = [None] * NCH2

            def s2_gq(c, gq):
                # one PSUM bank: groups gq*4 .. gq*4+3 for chunk c
                if w_ts[c] is None:
                    w_ts[c] = wpool.tile([128, 32 * 128], BF16, tag="w",
                                         name=f"w{c}")
                w_t = w_ts[c]
                wr = w_t[:].rearrange("p (i g s) -> p g i s", i=K, g=32, s=8)
                ps2 = ps2p.tile([128, 512], F32, tag="ps2")
                for gk in range(4):
                    g = gq * 4 + gk
                    nc.tensor.matmul(
                        ps2[:, gk * 128:(gk + 1) * 128],
                        u2_ap(g, c * TCH, (c + 1) * TCH),
                        mix_sb[:, g * 128:(g + 1) * 128],
                        start=True, stop=True,
                    )
                evac(wr[:, gq * 4:(gq + 1) * 4],
                     ps2[:].rearrange("p (g i s) -> p g i s", g=4, i=K))

            def s2(c):
                for gq in range(8):
                    s2_gq(c, gq)

            def t2(c):
                for grp in range(4):
                    pst = pstp.tile([128, 1024], BF16, tag="pst")
                    for k in range(8):
                        ik = grp * 8 + k     # i*2 + kt
                        nc.tensor.transpose(
                            pst[:, k * 128:(k + 1) * 128],
                            w_ts[c][:, ik * 128:(ik + 1) * 128],
                            id_sb[:],
                        )
                    evac(w_ts[c][:, grp * 1024:(grp + 1) * 1024], pst[:])

            def s3(c):
                # y staged in halves (smaller SBUF footprint); out on sync,
                # which is idle during the s2/t2/s3 phase.
                for yh in range(2):
                    y_t = ypool.tile([128, D // 2], BF16, tag="y")
                    for iph in range(4):
                        ip = yh * 4 + iph    # i-pair
                        ps3 = ps3p.tile([128, 512], F32, tag="ps3")
                        for ih in range(2):
                            i = ip * 2 + ih
                            for kt in range(2):
                                nc.tensor.matmul(
                                    ps3[:, ih * 256:(ih + 1) * 256],
                                    w_ts[c][:, (i * 2 + kt) * 128:(i * 2 + kt + 1) * 128],
                                    r_sb[:, kt * 256:(kt + 1) * 256],
                                    start=(kt == 0), stop=(kt == 1),
                                )
                        evac(y_t[:, iph * 512:(iph + 1) * 512], ps3[:])
                    nc.sync.dma_start(
                        yD[c * TCH:(c + 1) * TCH,
                           yh * (D // 2):(yh + 1) * (D // 2)], y_t[:])

            # Phase A: stage-2 gq-outer over the first 6 chunks, so the PE
            # consumes shuffle-1 groups in ring-arrival order instead of
            # stalling on chunk 0's last groups.  Phase B: t2/s3 sweep
            # (no shuffle dependency), with the last 2 chunks' stage-2
            # interleaved once their w buffers free up.
            NPRE = 6
            for gq in range(8):
                for c in range(NPRE):
                    s2_gq(c, gq)
            for c in range(NCH2):
                t2(c)
                s3(c)
                if c + NPRE < NCH2:
                    s2(c + NPRE)
    nc.compile()
    return nc


# ---------------------------------------------------------------- fft kernel

def _build_fft_nc():
    nc = bacc.Bacc("TRN2", target_bir_lowering=False, debug=False)
    # x_dev: row j*128+p, col tc*1024 + mt*512 + t  (4KB contiguous lines)
    xT = nc.dram_tensor("xT", [K * 128, 2 * NT], BF16, kind="ExternalInput").ap()
    tb_d = nc.dram_tensor("tb", [128, 512], BF16, kind="ExternalInput").ap()
    mix_d = nc.dram_tensor("mix", [128, 32 * 128], BF16, kind="ExternalInput").ap()
    r_d = nc.dram_tensor("rmat", [128, 512], BF16, kind="ExternalInput").ap()
    beta_d = nc.dram_tensor("beta", [128, 32], F32, kind="ExternalInput").ap()
    yT = nc.dram_tensor("yT", [D, NT], BF16, kind="ExternalOutput").ap()

    ec = [0]

    def evac(dst, src):
        # alternate PSUM->SBUF evacuation between DVE and ACT
        if ec[0] % 2 == 0:
            nc.vector.tensor_copy(dst, src)
        else:
            nc.scalar.copy(dst, src)
        ec[0] += 1

    def evac_act(dst, src):
        nc.scalar.copy(dst, src)

    sc = [0]
    shuf_mode = os.environ.get("KERNEL_SHUF", "hw")

    def shuffle_dma(dst, src):
        if shuf_mode == "hw":
            # HWDGE: alternate between the two HWDGE engines (sync, scalar)
            if sc[0] % 2 == 0:
                nc.sync.dma_start(dst, src)
            else:
                nc.scalar.dma_start(dst, src)
        else:
            # SWDGE: sbuf<->sbuf spreads across all 16 engines
            nc.gpsimd.dma_start(dst, src)
        sc[0] += 1

    NTH = NT // TC  # 512-token matmul halves within the full 1024 extent

    with tile.TileContext(nc) as tc:
        with (
            tc.tile_pool(name="consts", bufs=1) as consts,
            tc.tile_pool(name="xpool", bufs=3) as xpool,
            tc.tile_pool(name="upool", bufs=2) as upool,
            tc.tile_pool(name="u2pool", bufs=10) as u2pool,
            tc.tile_pool(name="v2pool", bufs=6) as v2pool,
            tc.tile_pool(name="vpool", bufs=1) as vpool,
            tc.tile_pool(name="ypool", bufs=3) as ypool,
            tc.tile_pool(name="psA", bufs=2, space="PSUM") as psA,
            tc.tile_pool(name="psB", bufs=3, space="PSUM") as psB,
            tc.tile_pool(name="psC", bufs=3, space="PSUM") as psC,
        ):
            tb_sb = consts.tile([128, 512], BF16)
            nc.sync.dma_start(tb_sb[:], tb_d[:])
            mix_sb = consts.tile([128, 32 * 128], BF16)
            nc.sync.dma_start(mix_sb[:], mix_d[:])
            r_sb = consts.tile([128, 512], BF16)
            nc.sync.dma_start(r_sb[:], r_d[:])
            beta_sb = consts.tile([128, 32], F32)
            nc.sync.dma_start(beta_sb[:], beta_d[:])

            # ---- stage 1: per block j, real-DFT projection (fp32r) ----
            u_sb = []
            for pb in range(2):
                u_pb = upool.tile([128, K * NT], BF16, tag="u")
                u_sb.append(u_pb)
            for j in range(K):
                x_t = xpool.tile([128, 2 * NT], BF16, tag="x")
                nc.sync.dma_start(x_t[:], xT[j * 128:(j + 1) * 128, :])
                for pb in range(2):
                    for th in range(NTH):
                        ps1 = psA.tile([128, TC], F32, tag="ps1")
                        for mt in range(2):
                            nc.tensor.matmul(
                                ps1[:],
                                tb_sb[:, (mt * 2 + pb) * 128:(mt * 2 + pb + 1) * 128],
                                x_t[:, mt * NT + th * TC:mt * NT + (th + 1) * TC],
                                start=(mt == 0), stop=(mt == 1),
                            )
                        evac_act(u_sb[pb][:, j * NT + th * TC:j * NT + (th + 1) * TC],
                                 ps1[:])

            # ---- per pb-half: shuffle1 -> stage 2 -> shuffle2 ----
            v_sb = vpool.tile([128, 32 * NT], BF16, tag="v")
            for pb in range(2):
                for gl in range(16):
                    G = 16 * pb + gl
                    u2_t = u2pool.tile([128, NT], BF16, tag="u2")
                    shuffle_dma(u2_t[:], u_sb[pb][8 * gl:8 * gl + 8, :])
                    v2_t = v2pool.tile([128, NT], BF16, tag="v2")
                    for th in range(NTH):
                        ps2 = psB.tile([128, TC], F32, tag="ps2")
                        nc.tensor.matmul(
                            ps2[:],
                            mix_sb[:, G * 128:(G + 1) * 128],
                            u2_t[:, th * TC:(th + 1) * TC],
                            start=True, stop=True,
                        )
                        nc.vector.tensor_scalar_add(
                            v2_t[:, th * TC:(th + 1) * TC], ps2[:],
                            beta_sb[:, G:G + 1])
                    kt = G // 16
                    shuffle_dma(
                        v_sb[8 * gl:8 * gl + 8, kt * 16 * NT:(kt + 1) * 16 * NT],
                        v2_t[:],
                    )

            # ---- stage 3: per output block i, inverse basis + bias ----
            for i in range(K):
                for nb in range(2):
                    ob = i * 2 + nb
                    y_t = ypool.tile([128, NT], BF16, tag="y")
                    for th in range(NTH):
                        ps3 = psC.tile([128, TC], F32, tag="ps3")
                        for kt in range(2):
                            nc.tensor.matmul(
                                ps3[:],
                                r_sb[:, (kt * 2 + nb) * 128:(kt * 2 + nb + 1) * 128],
                                v_sb[:, (kt * 16 + i) * NT + th * TC:
                                     (kt * 16 + i) * NT + (th + 1) * TC],
                                start=(kt == 0), stop=(kt == 1),
                            )
                        evac(y_t[:, th * TC:(th + 1) * TC], ps3[:])
                    nc.scalar.dma_start(yT[ob * 128:(ob + 1) * 128, :], y_t[:])
    nc.compile()
    return nc


# ---------------------------------------------------------------- dense kernel

def _build_dense_nc():
    nc = bacc.Bacc("TRN2", target_bir_lowering=False, debug=False)
    xT = nc.dram_tensor("xT", [D, NT], F32R, kind="ExternalInput").ap()
    m = nc.dram_tensor("m", [D, D], F32R, kind="ExternalInput").ap()
    bias = nc.dram_tensor("bias", [D], F32, kind="ExternalInput").ap()
    yT = nc.dram_tensor("yT", [D, NT], F32, kind="ExternalOutput").ap()

    KT = D // 128
    OB = D // 128
    TH = NT // 512

    with tile.TileContext(nc) as tc:
        with (
            tc.tile_pool(name="xpool", bufs=KT) as xpool,
            tc.tile_pool(name="mpool", bufs=3) as mpool,
            tc.tile_pool(name="bpool", bufs=1) as bpool,
            tc.tile_pool(name="opool", bufs=6) as opool,
            tc.tile_pool(name="psum", bufs=8, space="PSUM") as psum_pool,
        ):
            bias_t = bpool.tile([128, OB], F32)
            nc.sync.dma_start(bias_t[:], bias.rearrange("(c p) -> p c", p=128))

            xts = []
            for kt in range(KT):
                xt = xpool.tile([128, NT], F32R, tag="x")
                nc.sync.dma_start(xt[:], xT[kt * 128:(kt + 1) * 128, :])
                xts.append(xt)

            for ob in range(OB):
                mt = mpool.tile([128, KT * 128], F32R, tag="m")
                nc.sync.dma_start(
                    mt[:].rearrange("p (t o) -> p t o", t=KT),
                    m[:, ob * 128:(ob + 1) * 128].rearrange("(t p) o -> p t o", p=128),
                )
                for th in range(TH):
                    ps = psum_pool.tile([128, 512], F32)
                    for kt in range(KT):
                        nc.tensor.matmul(
                            ps[:],
                            mt[:, kt * 128:(kt + 1) * 128],
                            xts[kt][:, th * 512:(th + 1) * 512],
                            start=(kt == 0), stop=(kt == KT - 1),
                        )
                    osb = opool.tile([128, 512], F32, tag="o")
                    nc.vector.tensor_scalar_add(osb[:], ps[:], bias_t[:, ob:ob + 1])
                    nc.sync.dma_start(
                        yT[ob * 128:(ob + 1) * 128, th * 512:(th + 1) * 512], osb[:])
    nc.compile()
    return nc


# ---------------------------------------------------------------- entry point

def _run(nc, in_maps):
    global LAST_EXEC_NS
    trace = bool(os.environ.get("BASS_TRACE"))
    res = bass_utils.run_bass_kernel_spmd(
        nc, in_maps, list(range(N_CORES)), trace=trace,
        tmpdir=os.environ.get("BASS_TRACE_DIR") or None,
    )
    LAST_EXEC_NS = res.exec_time_ns
    return res


def kernel(x, W, d_bernoulli, bias):
    x = np.asarray(x, dtype=np.float32)
    W = np.asarray(W, dtype=np.float32)
    d_bernoulli = np.asarray(d_bernoulli, dtype=np.float32)
    bias = np.asarray(bias, dtype=np.float32)

    impl = os.environ.get("KERNEL_IMPL", "v2")
    xT = np.ascontiguousarray((x * d_bernoulli[None, :]).T)

    if impl == "v2":
        if "v2" not in _CACHE:
            _CACHE["v2"] = _build_v2_nc()
        t, mix, rd, ident = _v2_host_mats(W)
        xTb = xT.astype(BF16_NP)
        in_maps = []
        for c in range(N_CORES):
            xs = xTb[:, c * NT:(c + 1) * NT]                   # (D, NT)
            xd = (xs.reshape(K, 2, 128, NT)
                  .transpose(0, 2, 1, 3)
                  .reshape(K * 128, 2 * NT))
            in_maps.append({
                "xT": np.ascontiguousarray(xd),
                "tmat": t, "mix": mix, "rmat": rd, "ident": ident,
            })
        res = _run(_CACHE["v2"], in_maps)
        out = np.empty((B, D), dtype=np.float32)
        for c in range(N_CORES):
            out[c * NT:(c + 1) * NT, :] = \
                res.results[c]["yD"].astype(np.float32) + bias[None, :]
        return out

    if impl == "dense":
        if "dense" not in _CACHE:
            _CACHE["dense"] = _build_dense_nc()
        midx = (np.arange(BS)[:, None] - np.arange(BS)[None, :]) % BS
        M = np.empty((D, D), dtype=np.float32)
        for i in range(K):
            for j in range(K):
                M[j * BS:(j + 1) * BS, i * BS:(i + 1) * BS] = W[i, j][midx]
        in_maps = [
            {"xT": np.ascontiguousarray(xT[:, c * NT:(c + 1) * NT]),
             "m": M, "bias": bias}
            for c in range(N_CORES)
        ]
        res = _run(_CACHE["dense"], in_maps)
    else:
        if "fft" not in _CACHE:
            _CACHE["fft"] = _build_fft_nc()
        tb, mix, rd, beta = _fft_host_mats(W, bias)
        in_maps = []
        xTb = xT.astype(BF16_NP)
        for c in range(N_CORES):
            xs = xTb[:, c * NT:(c + 1) * NT]                   # (D, NT)
            # device layout: row j*128+p, col mt*NT + t  (4KB contiguous lines)
            xd = (xs.reshape(K, 2, 128, NT)
                  .transpose(0, 2, 1, 3)
                  .reshape(K * 128, 2 * NT))
            in_maps.append({
                "xT": np.ascontiguousarray(xd),
                "tb": tb, "mix": mix, "rmat": rd, "beta": beta,
            })
        res = _run(_CACHE["fft"], in_maps)

    out = np.empty((B, D), dtype=np.float32)
    for c in range(N_CORES):
        out[c * NT:(c + 1) * NT, :] = res.results[c]["yT"].T.astype(np.float32)
    return out



# revision 32
# speedup vs baseline: 1.0398x; 1.0398x over previous
"""Block-circulant matmul kernel for 8 Trainium2 NeuronCores.

Reference op (per token row x of shape (4096,)):
    y = (x*d) @ M + bias,  M[(j,m),(i,n)] = W[i,j,(m-n)%256]  (circulant blocks)

Implementation (default "fft"): real-DFT factorization executed in three
matmul stages per core, data-parallel over the batch (1024 tokens/core):
  stage1: per input block j, project onto the 256-col real DFT basis (fp32r)
  stage2: per frequency-group G (4 pair-slots), one 128x128 block-diag mix (bf16)
  stage3: per output block i, inverse real DFT basis + bias (bf16)
Between stages, two SBUF->SBUF partition-shuffle DMA passes regroup the
data (frequency-major <-> block-major).  ~7.6x fewer FLOPs than dense.

"dense" fallback: y^T = M^T x^T as a plain fp32r matmul.

Self-contained: shapes hardcoded; no sibling imports.
"""
import os
import sys

for _p in ("/root/.axon_site", "/root/.axon_site/_ro/trn_rl_repo", "/root/.axon_site/_ro/pypackages"):
    if _p not in sys.path:
        sys.path.append(_p)

import numpy as np
import ml_dtypes

import concourse.bass as bass
import concourse.tile as tile
from concourse import bacc, mybir
from concourse import bass_utils

N_CORES = 8
B = 8192
D = 4096
BS = 256
K = 16             # blocks per side
NSLOT = BS // 2    # 128 frequency pair-slots
NT = B // N_CORES  # tokens per core (1024)
TC = 512           # token chunk
NCH = NT // TC     # chunks (2)

F32 = mybir.dt.float32
F32R = mybir.dt.float32r
BF16 = mybir.dt.bfloat16
BF16_NP = ml_dtypes.bfloat16

LAST_EXEC_NS = None
_CACHE = {}


# ---------------------------------------------------------------- host math

def _canonical_mats(W):
    m = np.arange(BS)
    T = np.zeros((BS, BS), np.float64)
    T[:, 0] = 1.0
    T[:, 1] = (-1.0) ** m
    for f in range(1, NSLOT):
        T[:, 2 * f] = np.cos(2 * np.pi * f * m / BS)
        T[:, 2 * f + 1] = np.sin(2 * np.pi * f * m / BS)

    Wf = np.fft.fft(W.astype(np.float64), axis=-1)
    p = Wf.real
    q = -Wf.imag

    jj = np.arange(K)
    M_slot = np.zeros((NSLOT, 2 * K, 2 * K), np.float64)
    for f in range(1, NSLOT):
        pf, qf = p[:, :, f], q[:, :, f]          # [i, j]
        M_slot[f][np.ix_(2 * jj, 2 * jj)] = pf.T
        M_slot[f][np.ix_(2 * jj + 1, 2 * jj)] = qf.T
        M_slot[f][np.ix_(2 * jj, 2 * jj + 1)] = qf.T
        M_slot[f][np.ix_(2 * jj + 1, 2 * jj + 1)] = -pf.T
    M_slot[0][np.ix_(2 * jj, 2 * jj)] = p[:, :, 0].T
    M_slot[0][np.ix_(2 * jj + 1, 2 * jj + 1)] = p[:, :, NSLOT].T

    n = np.arange(BS)
    R = np.zeros((BS, BS), np.float64)
    R[0, :] = 1.0 / BS
    R[1, :] = ((-1.0) ** n) / BS
    for f in range(1, NSLOT):
        R[2 * f, :] = 2.0 / BS * np.cos(2 * np.pi * f * n / BS)
        R[2 * f + 1, :] = -2.0 / BS * np.sin(2 * np.pi * f * n / BS)
    return T, M_slot, R


def _fft_host_mats(W, bias):
    T, M_slot, R = _canonical_mats(W)
    p_idx = np.arange(128)

    # tb_dram (128, 4*128): [p, (mt*2+pb)*128+col] = T[mt*128+p, colmap(pb,col)]
    tb = np.zeros((128, 512), np.float32)
    for pb in range(2):
        slot = 64 * pb + 4 * (p_idx // 8) + (p_idx % 8) // 2
        c = p_idx % 2
        cols = 2 * slot + c                       # canonical comp per device col
        for mt in range(2):
            tb[:, (mt * 2 + pb) * 128:(mt * 2 + pb + 1) * 128] = \
                T[mt * 128:(mt + 1) * 128, :][:, cols]

    # mix_dram (128, 32*128) bf16: [row, G*128+col]
    mix = np.zeros((128, 32 * 128), np.float64)
    kk = np.arange(K)
    for G in range(32):
        MG = np.zeros((128, 128), np.float64)
        for r in range(4):
            blk = M_slot[4 * G + r]
            for c in range(2):
                for cp in range(2):
                    MG[np.ix_(16 * (2 * r + c) + kk, 16 * (2 * r + cp) + kk)] = \
                        blk[np.ix_(2 * kk + c, 2 * kk + cp)]
        mix[:, G * 128:(G + 1) * 128] = MG

    # r_dram (128, 4*128): [p, (kt*2+nb)*128+col] = R[rowmap(kt,p), nb*128+col]
    rd = np.zeros((128, 512), np.float64)
    for kt in range(2):
        gl = p_idx // 8
        q = (p_idx % 8) // 2
        c = p_idx % 2
        rows = 2 * (64 * kt + 4 * gl + q) + c
        for nb in range(2):
            rd[:, (kt * 2 + nb) * 128:(kt * 2 + nb + 1) * 128] = \
                R[rows, :][:, nb * 128:(nb + 1) * 128]

    # beta: per output block i solve R^T beta_i = bias_i; fold into stage-2
    # layout (128, 32) f32: [16*qc + i, G] = beta_i[2*(4G+q)+c], qc = 2q+c
    beta = np.zeros((128, 32), np.float64)
    RTinv = np.linalg.inv(R.T)
    for i in range(K):
        bi = RTinv @ bias[i * BS:(i + 1) * BS].astype(np.float64)
        for G in range(32):
            for q in range(4):
                for c in range(2):
                    qc = 2 * q + c
                    beta[16 * qc + i, G] = bi[2 * (4 * G + q) + c]
    return (tb.astype(BF16_NP),
            mix.astype(BF16_NP),
            rd.astype(BF16_NP),
            beta.astype(np.float32))


# ---------------------------------------------------------------- v2 host math

def _v2_host_mats(W):
    """Constants for the v2 kernel.

    Device component index c = g*8 + s for group g in [0,32), s in [0,8);
    slot = 4g + s//2, canonical column q = 2*slot + (s%2), matching
    _canonical_mats' T/R column/row order (0=DC, 1=Nyquist, 2f/2f+1=cos/sin).
    """
    T, M_slot, R = _canonical_mats(W)
    cdev = np.arange(256)
    g_of = cdev // 8
    s_of = cdev % 8
    q_of = 2 * (4 * g_of + s_of // 2) + (s_of % 2)

    # t_dev (128, 512): [p, (h*2+pb)*128 + ci] = T[h*128+p, q(pb*128+ci)]
    t = np.zeros((128, 512))
    for h in range(2):
        for pb in range(2):
            cols = q_of[pb * 128:(pb + 1) * 128]
            t[:, (h * 2 + pb) * 128:(h * 2 + pb + 1) * 128] = \
                T[h * 128:(h + 1) * 128, :][:, cols]

    # mix_dev (128, 32*128): row (j//8)*64 + s*8 + (j%8)  (u2's partition
    # layout, implied by the plain-AP shuffle DMA), col g*128 + i*8 + s2;
    # value (s//2 == s2//2) * M_slot[4g + s//2][2j + s%2, 2i + s2%2]
    mix = np.zeros((128, 32 * 128))
    jj = np.arange(K)
    prow = (jj[:, None] // 8) * 64 + np.arange(8)[None, :] * 8 + (jj[:, None] % 8)
    for g in range(32):
        for r in range(4):
            blk = M_slot[4 * g + r]
            for cb1 in range(2):
                for cb2 in range(2):
                    mix[np.ix_(prow[:, 2 * r + cb1],
                               g * 128 + 8 * jj + 2 * r + cb2)] = \
                        blk[np.ix_(2 * jj + cb1, 2 * jj + cb2)]

    # r_dev (128, 512): [cc, kt*256 + n] = R[q(kt*128+cc), n]
    rd = np.zeros((128, 512))
    for kt in range(2):
        rd[:, kt * 256:(kt + 1) * 256] = R[q_of[kt * 128:(kt + 1) * 128], :]

    ident = np.eye(128)
    return (t.astype(BF16_NP), mix.astype(BF16_NP), rd.astype(BF16_NP),
            ident.astype(BF16_NP))


# ---------------------------------------------------------------- v2 kernel

TCH = 128            # token chunk for stages 2/3 (transpose granularity)
NCH2 = NT // TCH     # 8 chunks


def _build_v2_nc():
    nc = bacc.Bacc("TRN2", target_bir_lowering=False, debug=False)
    # x_dev: row j*128+p, col h*NT + t  (4KB contiguous lines)
    xT = nc.dram_tensor("xT", [K * 128, 2 * NT], BF16, kind="ExternalInput").ap()
    t_d = nc.dram_tensor("tmat", [128, 512], BF16, kind="ExternalInput").ap()
    mix_d = nc.dram_tensor("mix", [128, 32 * 128], BF16, kind="ExternalInput").ap()
    r_d = nc.dram_tensor("rmat", [128, 512], BF16, kind="ExternalInput").ap()
    id_d = nc.dram_tensor("ident", [128, 128], BF16, kind="ExternalInput").ap()
    # y_dev: row tok (chunk*128+p), col feature  (8KB contiguous lines)
    yD = nc.dram_tensor("yD", [NT, D], BF16, kind="ExternalOutput").ap()

    ec = [0]
    n_evac = int(os.environ.get("KERNEL_NEVAC", "2"))

    def evac(dst, src):
        m = ec[0] % n_evac
        if m == 0:
            nc.vector.tensor_copy(dst, src)
        elif m == 1:
            nc.scalar.copy(dst, src)
        else:
            nc.gpsimd.tensor_copy(dst, src)
        ec[0] += 1

    with tile.TileContext(nc) as tc:
        with (
            tc.tile_pool(name="consts", bufs=1) as consts,
            tc.tile_pool(name="xpool", bufs=3) as xpool,
            tc.tile_pool(name="upool", bufs=1) as upool,
            tc.tile_pool(name="u2pool", bufs=1) as u2pool,
            tc.tile_pool(name="wpool", bufs=6) as wpool,
            tc.tile_pool(name="ypool", bufs=2) as ypool,
            tc.tile_pool(name="ps1p", bufs=2, space="PSUM") as ps1p,
            tc.tile_pool(name="ps2p", bufs=2, space="PSUM") as ps2p,
            tc.tile_pool(name="pstp", bufs=2, space="PSUM") as pstp,
            tc.tile_pool(name="ps3p", bufs=2, space="PSUM") as ps3p,
        ):
            # first stage-1 operands load before the big constants so the PE
            # can start as early as possible (t first: needed by the first mm)
            t_sb = consts.tile([128, 512], BF16)
            nc.sync.dma_start(t_sb[:], t_d[:])
            id_sb = consts.tile([128, 128], BF16)
            nc.scalar.dma_start(id_sb[:], id_d[:])
            mix_sb = consts.tile([128, 32 * 128], BF16)
            nc.scalar.dma_start(mix_sb[:], mix_d[:])
            r_sb = consts.tile([128, 512], BF16)
            nc.scalar.dma_start(r_sb[:], r_d[:])

            # ---- stage 1 (j-outer, x streamed): u[pb][c, j*NT+t] ----------
            u_sb = [upool.tile([128, K * NT], BF16, tag=f"u{pb}",
                               name=f"u{pb}") for pb in range(2)]
            # shuffle-1 output, split by queue ownership so no tensor is
            # written from two DMA queues: g<NSW -> SWDGE, then sync, scalar.
            NSW, NSY = 18, 7
            NSC = 32 - NSW - NSY
            u2a = u2pool.tile([128, NSW * NT], BF16, tag="u2a", name="u2a")
            u2b = u2pool.tile([128, NSY * NT], BF16, tag="u2b", name="u2b")
            u2c = u2pool.tile([128, NSC * NT], BF16, tag="u2c", name="u2c")

            def u2_ap(g, c0, c1):
                if g < NSW:
                    return u2a[:, g * NT + c0:g * NT + c1]
                if g < NSW + NSY:
                    gg = g - NSW
                    return u2b[:, gg * NT + c0:gg * NT + c1]
                gg = g - NSW - NSY
                return u2c[:, gg * NT + c0:gg * NT + c1]

            def sh1(jh):
                # u2_g[jh*64 + s*8 + jl, t] = u[pb][gl*8+s, (jh*8+jl)*NT+t].
                # Plain 2D APs on both sides (partition map implied by the
                # DMA balancer); full-NT runs (2KB packets). SWDGE only:
                # HWDGE triggers block the sync/scalar streams and starve
                # x-loads/evacs (measured), gpsimd has nothing else to do.
                for g in range(32):
                    pb, gl = g // 16, g % 16
                    dst = u2_ap(g, 0, NT)[jh * 64:(jh + 1) * 64, :]
                    src = u_sb[pb][8 * gl:8 * gl + 8,
                                   jh * 8 * NT:(jh + 1) * 8 * NT]
                    nc.gpsimd.dma_start(dst, src)

            for j in range(K):
                x_t = xpool.tile([128, 2 * NT], BF16, tag="x")
                if j % 2 == 0:
                    nc.sync.dma_start(x_t[:], xT[j * 128:(j + 1) * 128, :])
                else:
                    nc.scalar.dma_start(x_t[:], xT[j * 128:(j + 1) * 128, :])
                for pb in range(2):
                    for th in range(2):
                        ps1 = ps1p.tile([128, 512], F32, tag="ps1")
                        for h in range(2):
                            nc.tensor.matmul(
                                ps1[:],
                                t_sb[:, (h * 2 + pb) * 128:(h * 2 + pb + 1) * 128],
                                x_t[:, h * NT + th * 512:h * NT + (th + 1) * 512],
                                start=(h == 0), stop=(h == 1),
                            )
                        evac(u_sb[pb][:, j * NT + th * 512:j * NT + (th + 1) * 512],
                             ps1[:])
                if j == K // 2 - 1:
                    sh1(0)
            sh1(1)

            # ---- stages 2/3 per 128-token chunk, software-pipelined --------
            # w is i-major (col = i*256 + g*8 + s2); stage-2 packs 4 groups
            # per PSUM bank and evacuates with one 4D-scatter copy. T2 reads
            # contiguous ik blocks and writes v back in place (disjoint).
            w_ts = [None] * NCH2

            def s2_gq(c, gq):
                # one PSUM bank: groups gq*4 .. gq*4+3 for chunk c
                if w_ts[c] is None:
                    w_ts[c] = wpool.tile([128, 32 * 128], BF16, tag="w",
                                         name=f"w{c}")
                w_t = w_ts[c]
                wr = w_t[:].rearrange("p (i g s) -> p g i s", i=K, g=32, s=8)
                ps2 = ps2p.tile([128, 512], F32, tag="ps2")
                for gk in range(4):
                    g = gq * 4 + gk
                    nc.tensor.matmul(
                        ps2[:, gk * 128:(gk + 1) * 128],
                        u2_ap(g, c * TCH, (c + 1) * TCH),
                        mix_sb[:, g * 128:(g + 1) * 128],
                        start=True, stop=True,
                    )
                evac(wr[:, gq * 4:(gq + 1) * 4],
                     ps2[:].rearrange("p (g i s) -> p g i s", g=4, i=K))

            def s2(c):
                for gq in range(8):
                    s2_gq(c, gq)

            def t2(c):
                for grp in range(4):
                    pst = pstp.tile([128, 1024], BF16, tag="pst")
                    for k in range(8):
                        ik = grp * 8 + k     # i*2 + kt
                        nc.tensor.transpose(
                            pst[:, k * 128:(k + 1) * 128],
                            w_ts[c][:, ik * 128:(ik + 1) * 128],
                            id_sb[:],
                        )
                    evac(w_ts[c][:, grp * 1024:(grp + 1) * 1024], pst[:])

            def s3(c):
                # y staged in halves (smaller SBUF footprint); out on sync,
                # which is idle during the s2/t2/s3 phase.
                for yh in range(2):
                    y_t = ypool.tile([128, D // 2], BF16, tag="y")
                    for iph in range(4):
                        ip = yh * 4 + iph    # i-pair
                        ps3 = ps3p.tile([128, 512], F32, tag="ps3")
                        for ih in range(2):
                            i = ip * 2 + ih
                            for kt in range(2):
                                nc.tensor.matmul(
                                    ps3[:, ih * 256:(ih + 1) * 256],
                                    w_ts[c][:, (i * 2 + kt) * 128:(i * 2 + kt + 1) * 128],
                                    r_sb[:, kt * 256:(kt + 1) * 256],
                                    start=(kt == 0), stop=(kt == 1),
                                )
                        evac(y_t[:, iph * 512:(iph + 1) * 512], ps3[:])
                    nc.sync.dma_start(
                        yD[c * TCH:(c + 1) * TCH,
                           yh * (D // 2):(yh + 1) * (D // 2)], y_t[:])

            # Phase A: stage-2 gq-outer over the first 6 chunks, so the PE
            # consumes shuffle-1 groups in ring-arrival order instead of
            # stalling on chunk 0's last groups.  Phase B: t2/s3 sweep
            # (no shuffle dependency), with the last 2 chunks' stage-2
            # interleaved once their w buffers free up.
            NPRE = 6
            for gq in range(8):
                for c in range(NPRE):
                    s2_gq(c, gq)
            for it in range(NCH2 + 1):
                if it < NCH2:
                    t2(it)
                if it >= 1:
                    s3(it - 1)
                    if it - 1 + NPRE < NCH2:
                        s2(it - 1 + NPRE)
    nc.compile()
    return nc


# ---------------------------------------------------------------- fft kernel

def _build_fft_nc():
    nc = bacc.Bacc("TRN2", target_bir_lowering=False, debug=False)
    # x_dev: row j*128+p, col tc*1024 + mt*512 + t  (4KB contiguous lines)
    xT = nc.dram_tensor("xT", [K * 128, 2 * NT], BF16, kind="ExternalInput").ap()
    tb_d = nc.dram_tensor("tb", [128, 512], BF16, kind="ExternalInput").ap()
    mix_d = nc.dram_tensor("mix", [128, 32 * 128], BF16, kind="ExternalInput").ap()
    r_d = nc.dram_tensor("rmat", [128, 512], BF16, kind="ExternalInput").ap()
    beta_d = nc.dram_tensor("beta", [128, 32], F32, kind="ExternalInput").ap()
    yT = nc.dram_tensor("yT", [D, NT], BF16, kind="ExternalOutput").ap()

    ec = [0]

    def evac(dst, src):
        # alternate PSUM->SBUF evacuation between DVE and ACT
        if ec[0] % 2 == 0:
            nc.vector.tensor_copy(dst, src)
        else:
            nc.scalar.copy(dst, src)
        ec[0] += 1

    def evac_act(dst, src):
        nc.scalar.copy(dst, src)

    sc = [0]
    shuf_mode = os.environ.get("KERNEL_SHUF", "hw")

    def shuffle_dma(dst, src):
        if shuf_mode == "hw":
            # HWDGE: alternate between the two HWDGE engines (sync, scalar)
            if sc[0] % 2 == 0:
                nc.sync.dma_start(dst, src)
            else:
                nc.scalar.dma_start(dst, src)
        else:
            # SWDGE: sbuf<->sbuf spreads across all 16 engines
            nc.gpsimd.dma_start(dst, src)
        sc[0] += 1

    NTH = NT // TC  # 512-token matmul halves within the full 1024 extent

    with tile.TileContext(nc) as tc:
        with (
            tc.tile_pool(name="consts", bufs=1) as consts,
            tc.tile_pool(name="xpool", bufs=3) as xpool,
            tc.tile_pool(name="upool", bufs=2) as upool,
            tc.tile_pool(name="u2pool", bufs=10) as u2pool,
            tc.tile_pool(name="v2pool", bufs=6) as v2pool,
            tc.tile_pool(name="vpool", bufs=1) as vpool,
            tc.tile_pool(name="ypool", bufs=3) as ypool,
            tc.tile_pool(name="psA", bufs=2, space="PSUM") as psA,
            tc.tile_pool(name="psB", bufs=3, space="PSUM") as psB,
            tc.tile_pool(name="psC", bufs=3, space="PSUM") as psC,
        ):
            tb_sb = consts.tile([128, 512], BF16)
            nc.sync.dma_start(tb_sb[:], tb_d[:])
            mix_sb = consts.tile([128, 32 * 128], BF16)
            nc.sync.dma_start(mix_sb[:], mix_d[:])
            r_sb = consts.tile([128, 512], BF16)
            nc.sync.dma_start(r_sb[:], r_d[:])
            beta_sb = consts.tile([128, 32], F32)
            nc.sync.dma_start(beta_sb[:], beta_d[:])

            # ---- stage 1: per block j, real-DFT projection (fp32r) ----
            u_sb = []
            for pb in range(2):
                u_pb = upool.tile([128, K * NT], BF16, tag="u")
                u_sb.append(u_pb)
            for j in range(K):
                x_t = xpool.tile([128, 2 * NT], BF16, tag="x")
                nc.sync.dma_start(x_t[:], xT[j * 128:(j + 1) * 128, :])
                for pb in range(2):
                    for th in range(NTH):
                        ps1 = psA.tile([128, TC], F32, tag="ps1")
                        for mt in range(2):
                            nc.tensor.matmul(
                                ps1[:],
                                tb_sb[:, (mt * 2 + pb) * 128:(mt * 2 + pb + 1) * 128],
                                x_t[:, mt * NT + th * TC:mt * NT + (th + 1) * TC],
                                start=(mt == 0), stop=(mt == 1),
                            )
                        evac_act(u_sb[pb][:, j * NT + th * TC:j * NT + (th + 1) * TC],
                                 ps1[:])

            # ---- per pb-half: shuffle1 -> stage 2 -> shuffle2 ----
            v_sb = vpool.tile([128, 32 * NT], BF16, tag="v")
            for pb in range(2):
                for gl in range(16):
                    G = 16 * pb + gl
                    u2_t = u2pool.tile([128, NT], BF16, tag="u2")
                    shuffle_dma(u2_t[:], u_sb[pb][8 * gl:8 * gl + 8, :])
                    v2_t = v2pool.tile([128, NT], BF16, tag="v2")
                    for th in range(NTH):
                        ps2 = psB.tile([128, TC], F32, tag="ps2")
                        nc.tensor.matmul(
                            ps2[:],
                            mix_sb[:, G * 128:(G + 1) * 128],
                            u2_t[:, th * TC:(th + 1) * TC],
                            start=True, stop=True,
                        )
                        nc.vector.tensor_scalar_add(
                            v2_t[:, th * TC:(th + 1) * TC], ps2[:],
                            beta_sb[:, G:G + 1])
                    kt = G // 16
                    shuffle_dma(
                        v_sb[8 * gl:8 * gl + 8, kt * 16 * NT:(kt + 1) * 16 * NT],
                        v2_t[:],
                    )

            # ---- stage 3: per output block i, inverse basis + bias ----
            for i in range(K):
                for nb in range(2):
                    ob = i * 2 + nb
                    y_t = ypool.tile([128, NT], BF16, tag="y")
                    for th in range(NTH):
                        ps3 = psC.tile([128, TC], F32, tag="ps3")
                        for kt in range(2):
                            nc.tensor.matmul(
                                ps3[:],
                                r_sb[:, (kt * 2 + nb) * 128:(kt * 2 + nb + 1) * 128],
                                v_sb[:, (kt * 16 + i) * NT + th * TC:
                                     (kt * 16 + i) * NT + (th + 1) * TC],
                                start=(kt == 0), stop=(kt == 1),
                            )
                        evac(y_t[:, th * TC:(th + 1) * TC], ps3[:])
                    nc.scalar.dma_start(yT[ob * 128:(ob + 1) * 128, :], y_t[:])
    nc.compile()
    return nc


# ---------------------------------------------------------------- dense kernel

def _build_dense_nc():
    nc = bacc.Bacc("TRN2", target_bir_lowering=False, debug=False)
    xT = nc.dram_tensor("xT", [D, NT], F32R, kind="ExternalInput").ap()
    m = nc.dram_tensor("m", [D, D], F32R, kind="ExternalInput").ap()
    bias = nc.dram_tensor("bias", [D], F32, kind="ExternalInput").ap()
    yT = nc.dram_tensor("yT", [D, NT], F32, kind="ExternalOutput").ap()

    KT = D // 128
    OB = D // 128
    TH = NT // 512

    with tile.TileContext(nc) as tc:
        with (
            tc.tile_pool(name="xpool", bufs=KT) as xpool,
            tc.tile_pool(name="mpool", bufs=3) as mpool,
            tc.tile_pool(name="bpool", bufs=1) as bpool,
            tc.tile_pool(name="opool", bufs=6) as opool,
            tc.tile_pool(name="psum", bufs=8, space="PSUM") as psum_pool,
        ):
            bias_t = bpool.tile([128, OB], F32)
            nc.sync.dma_start(bias_t[:], bias.rearrange("(c p) -> p c", p=128))

            xts = []
            for kt in range(KT):
                xt = xpool.tile([128, NT], F32R, tag="x")
                nc.sync.dma_start(xt[:], xT[kt * 128:(kt + 1) * 128, :])
                xts.append(xt)

            for ob in range(OB):
                mt = mpool.tile([128, KT * 128], F32R, tag="m")
                nc.sync.dma_start(
                    mt[:].rearrange("p (t o) -> p t o", t=KT),
                    m[:, ob * 128:(ob + 1) * 128].rearrange("(t p) o -> p t o", p=128),
                )
                for th in range(TH):
                    ps = psum_pool.tile([128, 512], F32)
                    for kt in range(KT):
                        nc.tensor.matmul(
                            ps[:],
                            mt[:, kt * 128:(kt + 1) * 128],
                            xts[kt][:, th * 512:(th + 1) * 512],
                            start=(kt == 0), stop=(kt == KT - 1),
                        )
                    osb = opool.tile([128, 512], F32, tag="o")
                    nc.vector.tensor_scalar_add(osb[:], ps[:], bias_t[:, ob:ob + 1])
                    nc.sync.dma_start(
                        yT[ob * 128:(ob + 1) * 128, th * 512:(th + 1) * 512], osb[:])
    nc.compile()
    return nc


# ---------------------------------------------------------------- entry point

def _run(nc, in_maps):
    global LAST_EXEC_NS
    trace = bool(os.environ.get("BASS_TRACE"))
    res = bass_utils.run_bass_kernel_spmd(
        nc, in_maps, list(range(N_CORES)), trace=trace,
        tmpdir=os.environ.get("BASS_TRACE_DIR") or None,
    )
    LAST_EXEC_NS = res.exec_time_ns
    return res


def kernel(x, W, d_bernoulli, bias):
    x = np.asarray(x, dtype=np.float32)
    W = np.asarray(W, dtype=np.float32)
    d_bernoulli = np.asarray(d_bernoulli, dtype=np.float32)
    bias = np.asarray(bias, dtype=np.float32)

    impl = os.environ.get("KERNEL_IMPL", "v2")
    xT = np.ascontiguousarray((x * d_bernoulli[None, :]).T)

    if impl == "v2":
        if "v2" not in _CACHE:
            _CACHE["v2"] = _build_v2_nc()
        t, mix, rd, ident = _v2_host_mats(W)
        xTb = xT.astype(BF16_NP)
        in_maps = []
        for c in range(N_CORES):
            xs = xTb[:, c * NT:(c + 1) * NT]                   # (D, NT)
            xd = (xs.reshape(K, 2, 128, NT)
                  .transpose(0, 2, 1, 3)
                  .reshape(K * 128, 2 * NT))
            in_maps.append({
                "xT": np.ascontiguousarray(xd),
                "tmat": t, "mix": mix, "rmat": rd, "ident": ident,
            })
        res = _run(_CACHE["v2"], in_maps)
        out = np.empty((B, D), dtype=np.float32)
        for c in range(N_CORES):
            out[c * NT:(c + 1) * NT, :] = \
                res.results[c]["yD"].astype(np.float32) + bias[None, :]
        return out

    if impl == "dense":
        if "dense" not in _CACHE:
            _CACHE["dense"] = _build_dense_nc()
        midx = (np.arange(BS)[:, None] - np.arange(BS)[None, :]) % BS
        M = np.empty((D, D), dtype=np.float32)
        for i in range(K):
            for j in range(K):
                M[j * BS:(j + 1) * BS, i * BS:(i + 1) * BS] = W[i, j][midx]
        in_maps = [
            {"xT": np.ascontiguousarray(xT[:, c * NT:(c + 1) * NT]),
             "m": M, "bias": bias}
            for c in range(N_CORES)
        ]
        res = _run(_CACHE["dense"], in_maps)
    else:
        if "fft" not in _CACHE:
            _CACHE["fft"] = _build_fft_nc()
        tb, mix, rd, beta = _fft_host_mats(W, bias)
        in_maps = []
        xTb = xT.astype(BF16_NP)
        for c in range(N_CORES):
            xs = xTb[:, c * NT:(c + 1) * NT]                   # (D, NT)
            # device layout: row j*128+p, col mt*NT + t  (4KB contiguous lines)
            xd = (xs.reshape(K, 2, 128, NT)
                  .transpose(0, 2, 1, 3)
                  .reshape(K * 128, 2 * NT))
            in_maps.append({
                "xT": np.ascontiguousarray(xd),
                "tb": tb, "mix": mix, "rmat": rd, "beta": beta,
            })
        res = _run(_CACHE["fft"], in_maps)

    out = np.empty((B, D), dtype=np.float32)
    for c in range(N_CORES):
        out[c * NT:(c + 1) * NT, :] = res.results[c]["yT"].T.astype(np.float32)
    return out



# revision 33
# speedup vs baseline: 1.0652x; 1.0245x over previous
"""Block-circulant matmul kernel for 8 Trainium2 NeuronCores.

Reference op (per token row x of shape (4096,)):
    y = (x*d) @ M + bias,  M[(j,m),(i,n)] = W[i,j,(m-n)%256]  (circulant blocks)

Implementation (default "fft"): real-DFT factorization executed in three
matmul stages per core, data-parallel over the batch (1024 tokens/core):
  stage1: per input block j, project onto the 256-col real DFT basis (fp32r)
  stage2: per frequency-group G (4 pair-slots), one 128x128 block-diag mix (bf16)
  stage3: per output block i, inverse real DFT basis + bias (bf16)
Between stages, two SBUF->SBUF partition-shuffle DMA passes regroup the
data (frequency-major <-> block-major).  ~7.6x fewer FLOPs than dense.

"dense" fallback: y^T = M^T x^T as a plain fp32r matmul.

Self-contained: shapes hardcoded; no sibling imports.
"""
import os
import sys

for _p in ("/root/.axon_site", "/root/.axon_site/_ro/trn_rl_repo", "/root/.axon_site/_ro/pypackages"):
    if _p not in sys.path:
        sys.path.append(_p)

import numpy as np
import ml_dtypes

import concourse.bass as bass
import concourse.tile as tile
from concourse import bacc, mybir
from concourse import bass_utils

N_CORES = 8
B = 8192
D = 4096
BS = 256
K = 16             # blocks per side
NSLOT = BS // 2    # 128 frequency pair-slots
NT = B // N_CORES  # tokens per core (1024)
TC = 512           # token chunk
NCH = NT // TC     # chunks (2)

F32 = mybir.dt.float32
F32R = mybir.dt.float32r
BF16 = mybir.dt.bfloat16
BF16_NP = ml_dtypes.bfloat16

LAST_EXEC_NS = None
_CACHE = {}


# ---------------------------------------------------------------- host math

def _canonical_mats(W):
    m = np.arange(BS)
    T = np.zeros((BS, BS), np.float64)
    T[:, 0] = 1.0
    T[:, 1] = (-1.0) ** m
    for f in range(1, NSLOT):
        T[:, 2 * f] = np.cos(2 * np.pi * f * m / BS)
        T[:, 2 * f + 1] = np.sin(2 * np.pi * f * m / BS)

    Wf = np.fft.fft(W.astype(np.float64), axis=-1)
    p = Wf.real
    q = -Wf.imag

    jj = np.arange(K)
    M_slot = np.zeros((NSLOT, 2 * K, 2 * K), np.float64)
    for f in range(1, NSLOT):
        pf, qf = p[:, :, f], q[:, :, f]          # [i, j]
        M_slot[f][np.ix_(2 * jj, 2 * jj)] = pf.T
        M_slot[f][np.ix_(2 * jj + 1, 2 * jj)] = qf.T
        M_slot[f][np.ix_(2 * jj, 2 * jj + 1)] = qf.T
        M_slot[f][np.ix_(2 * jj + 1, 2 * jj + 1)] = -pf.T
    M_slot[0][np.ix_(2 * jj, 2 * jj)] = p[:, :, 0].T
    M_slot[0][np.ix_(2 * jj + 1, 2 * jj + 1)] = p[:, :, NSLOT].T

    n = np.arange(BS)
    R = np.zeros((BS, BS), np.float64)
    R[0, :] = 1.0 / BS
    R[1, :] = ((-1.0) ** n) / BS
    for f in range(1, NSLOT):
        R[2 * f, :] = 2.0 / BS * np.cos(2 * np.pi * f * n / BS)
        R[2 * f + 1, :] = -2.0 / BS * np.sin(2 * np.pi * f * n / BS)
    return T, M_slot, R


def _fft_host_mats(W, bias):
    T, M_slot, R = _canonical_mats(W)
    p_idx = np.arange(128)

    # tb_dram (128, 4*128): [p, (mt*2+pb)*128+col] = T[mt*128+p, colmap(pb,col)]
    tb = np.zeros((128, 512), np.float32)
    for pb in range(2):
        slot = 64 * pb + 4 * (p_idx // 8) + (p_idx % 8) // 2
        c = p_idx % 2
        cols = 2 * slot + c                       # canonical comp per device col
        for mt in range(2):
            tb[:, (mt * 2 + pb) * 128:(mt * 2 + pb + 1) * 128] = \
                T[mt * 128:(mt + 1) * 128, :][:, cols]

    # mix_dram (128, 32*128) bf16: [row, G*128+col]
    mix = np.zeros((128, 32 * 128), np.float64)
    kk = np.arange(K)
    for G in range(32):
        MG = np.zeros((128, 128), np.float64)
        for r in range(4):
            blk = M_slot[4 * G + r]
            for c in range(2):
                for cp in range(2):
                    MG[np.ix_(16 * (2 * r + c) + kk, 16 * (2 * r + cp) + kk)] = \
                        blk[np.ix_(2 * kk + c, 2 * kk + cp)]
        mix[:, G * 128:(G + 1) * 128] = MG

    # r_dram (128, 4*128): [p, (kt*2+nb)*128+col] = R[rowmap(kt,p), nb*128+col]
    rd = np.zeros((128, 512), np.float64)
    for kt in range(2):
        gl = p_idx // 8
        q = (p_idx % 8) // 2
        c = p_idx % 2
        rows = 2 * (64 * kt + 4 * gl + q) + c
        for nb in range(2):
            rd[:, (kt * 2 + nb) * 128:(kt * 2 + nb + 1) * 128] = \
                R[rows, :][:, nb * 128:(nb + 1) * 128]

    # beta: per output block i solve R^T beta_i = bias_i; fold into stage-2
    # layout (128, 32) f32: [16*qc + i, G] = beta_i[2*(4G+q)+c], qc = 2q+c
    beta = np.zeros((128, 32), np.float64)
    RTinv = np.linalg.inv(R.T)
    for i in range(K):
        bi = RTinv @ bias[i * BS:(i + 1) * BS].astype(np.float64)
        for G in range(32):
            for q in range(4):
                for c in range(2):
                    qc = 2 * q + c
                    beta[16 * qc + i, G] = bi[2 * (4 * G + q) + c]
    return (tb.astype(BF16_NP),
            mix.astype(BF16_NP),
            rd.astype(BF16_NP),
            beta.astype(np.float32))


# ---------------------------------------------------------------- v2 host math

def _v2_host_mats(W):
    """Constants for the v2 kernel.

    Device component index c = g*8 + s for group g in [0,32), s in [0,8);
    slot = 4g + s//2, canonical column q = 2*slot + (s%2), matching
    _canonical_mats' T/R column/row order (0=DC, 1=Nyquist, 2f/2f+1=cos/sin).
    """
    T, M_slot, R = _canonical_mats(W)
    cdev = np.arange(256)
    g_of = cdev // 8
    s_of = cdev % 8
    q_of = 2 * (4 * g_of + s_of // 2) + (s_of % 2)

    # t_dev (128, 512): [p, (h*2+pb)*128 + ci] = T[h*128+p, q(pb*128+ci)]
    t = np.zeros((128, 512))
    for h in range(2):
        for pb in range(2):
            cols = q_of[pb * 128:(pb + 1) * 128]
            t[:, (h * 2 + pb) * 128:(h * 2 + pb + 1) * 128] = \
                T[h * 128:(h + 1) * 128, :][:, cols]

    # mix_dev (128, 32*128): row (j//8)*64 + s*8 + (j%8)  (u2's partition
    # layout, implied by the plain-AP shuffle DMA), col g*128 + i*8 + s2;
    # value (s//2 == s2//2) * M_slot[4g + s//2][2j + s%2, 2i + s2%2]
    mix = np.zeros((128, 32 * 128))
    jj = np.arange(K)
    prow = (jj[:, None] // 8) * 64 + np.arange(8)[None, :] * 8 + (jj[:, None] % 8)
    for g in range(32):
        for r in range(4):
            blk = M_slot[4 * g + r]
            for cb1 in range(2):
                for cb2 in range(2):
                    mix[np.ix_(prow[:, 2 * r + cb1],
                               g * 128 + 8 * jj + 2 * r + cb2)] = \
                        blk[np.ix_(2 * jj + cb1, 2 * jj + cb2)]

    # r_dev (128, 512): [cc, kt*256 + n] = R[q(kt*128+cc), n]
    rd = np.zeros((128, 512))
    for kt in range(2):
        rd[:, kt * 256:(kt + 1) * 256] = R[q_of[kt * 128:(kt + 1) * 128], :]

    ident = np.eye(128)
    return (t.astype(BF16_NP), mix.astype(BF16_NP), rd.astype(BF16_NP),
            ident.astype(BF16_NP))


# ---------------------------------------------------------------- v2 kernel

TCH = 128            # token chunk for stages 2/3 (transpose granularity)
NCH2 = NT // TCH     # 8 chunks


def _build_v2_nc():
    nc = bacc.Bacc("TRN2", target_bir_lowering=False, debug=False)
    # x_dev: row j*128+p, col h*NT + t  (4KB contiguous lines)
    xT = nc.dram_tensor("xT", [K * 128, 2 * NT], BF16, kind="ExternalInput").ap()
    t_d = nc.dram_tensor("tmat", [128, 512], BF16, kind="ExternalInput").ap()
    mix_d = nc.dram_tensor("mix", [128, 32 * 128], BF16, kind="ExternalInput").ap()
    r_d = nc.dram_tensor("rmat", [128, 512], BF16, kind="ExternalInput").ap()
    id_d = nc.dram_tensor("ident", [128, 128], BF16, kind="ExternalInput").ap()
    # y_dev: row tok (chunk*128+p), col feature  (8KB contiguous lines)
    yD = nc.dram_tensor("yD", [NT, D], BF16, kind="ExternalOutput").ap()

    ec = [0]
    n_evac = int(os.environ.get("KERNEL_NEVAC", "2"))

    def evac(dst, src):
        m = ec[0] % n_evac
        if m == 0:
            nc.vector.tensor_copy(dst, src)
        elif m == 1:
            nc.scalar.copy(dst, src)
        else:
            nc.gpsimd.tensor_copy(dst, src)
        ec[0] += 1

    with tile.TileContext(nc) as tc:
        with (
            tc.tile_pool(name="consts", bufs=1) as consts,
            tc.tile_pool(name="xpool", bufs=3) as xpool,
            tc.tile_pool(name="upool", bufs=1) as upool,
            tc.tile_pool(name="u2pool", bufs=1) as u2pool,
            tc.tile_pool(name="wpool", bufs=6) as wpool,
            tc.tile_pool(name="ypool", bufs=2) as ypool,
            tc.tile_pool(name="ps1p", bufs=2, space="PSUM") as ps1p,
            tc.tile_pool(name="ps2p", bufs=2, space="PSUM") as ps2p,
            tc.tile_pool(name="pstp", bufs=2, space="PSUM") as pstp,
            tc.tile_pool(name="ps3p", bufs=2, space="PSUM") as ps3p,
        ):
            # first stage-1 operands load before the big constants so the PE
            # can start as early as possible (t first: needed by the first mm)
            t_sb = consts.tile([128, 512], BF16)
            nc.sync.dma_start(t_sb[:], t_d[:])
            id_sb = consts.tile([128, 128], BF16)
            nc.scalar.dma_start(id_sb[:], id_d[:])
            mix_sb = consts.tile([128, 32 * 128], BF16)
            nc.scalar.dma_start(mix_sb[:], mix_d[:])
            r_sb = consts.tile([128, 512], BF16)
            nc.scalar.dma_start(r_sb[:], r_d[:])

            # ---- stage 1 (j-outer, x streamed): u[pb][c, j*NT+t] ----------
            u_sb = [upool.tile([128, K * NT], BF16, tag=f"u{pb}",
                               name=f"u{pb}") for pb in range(2)]
            # shuffle-1 output, split by queue ownership so no tensor is
            # written from two DMA queues: g<NSW -> SWDGE, then sync, scalar.
            NSW, NSY = 18, 7
            NSC = 32 - NSW - NSY
            u2a = u2pool.tile([128, NSW * NT], BF16, tag="u2a", name="u2a")
            u2b = u2pool.tile([128, NSY * NT], BF16, tag="u2b", name="u2b")
            u2c = u2pool.tile([128, NSC * NT], BF16, tag="u2c", name="u2c")

            def u2_ap(g, c0, c1):
                if g < NSW:
                    return u2a[:, g * NT + c0:g * NT + c1]
                if g < NSW + NSY:
                    gg = g - NSW
                    return u2b[:, gg * NT + c0:gg * NT + c1]
                gg = g - NSW - NSY
                return u2c[:, gg * NT + c0:gg * NT + c1]

            def sh1(jh):
                # u2_g[jh*64 + s*8 + jl, t] = u[pb][gl*8+s, (jh*8+jl)*NT+t].
                # Plain 2D APs on both sides (partition map implied by the
                # DMA balancer); full-NT runs (2KB packets). SWDGE only:
                # HWDGE triggers block the sync/scalar streams and starve
                # x-loads/evacs (measured), gpsimd has nothing else to do.
                for g in range(32):
                    pb, gl = g // 16, g % 16
                    dst = u2_ap(g, 0, NT)[jh * 64:(jh + 1) * 64, :]
                    src = u_sb[pb][8 * gl:8 * gl + 8,
                                   jh * 8 * NT:(jh + 1) * 8 * NT]
                    nc.gpsimd.dma_start(dst, src)

            for j in range(K):
                x_t = xpool.tile([128, 2 * NT], BF16, tag="x")
                # all x on sync: scalar must stay free for evacuations, a
                # single x-load between two ACT evacs stalls the ps1 ring
                nc.sync.dma_start(x_t[:], xT[j * 128:(j + 1) * 128, :])
                for pb in range(2):
                    for th in range(2):
                        ps1 = ps1p.tile([128, 512], F32, tag="ps1")
                        for h in range(2):
                            nc.tensor.matmul(
                                ps1[:],
                                t_sb[:, (h * 2 + pb) * 128:(h * 2 + pb + 1) * 128],
                                x_t[:, h * NT + th * 512:h * NT + (th + 1) * 512],
                                start=(h == 0), stop=(h == 1),
                            )
                        evac(u_sb[pb][:, j * NT + th * 512:j * NT + (th + 1) * 512],
                             ps1[:])
                if j == K // 2 - 1:
                    sh1(0)
            sh1(1)

            # ---- stages 2/3 per 128-token chunk, software-pipelined --------
            # w is i-major (col = i*256 + g*8 + s2); stage-2 packs 4 groups
            # per PSUM bank and evacuates with one 4D-scatter copy. T2 reads
            # contiguous ik blocks and writes v back in place (disjoint).
            w_ts = [None] * NCH2

            def s2_gq(c, gq):
                # one PSUM bank: groups gq*4 .. gq*4+3 for chunk c
                if w_ts[c] is None:
                    w_ts[c] = wpool.tile([128, 32 * 128], BF16, tag="w",
                                         name=f"w{c}")
                w_t = w_ts[c]
                wr = w_t[:].rearrange("p (i g s) -> p g i s", i=K, g=32, s=8)
                ps2 = ps2p.tile([128, 512], F32, tag="ps2")
                for gk in range(4):
                    g = gq * 4 + gk
                    nc.tensor.matmul(
                        ps2[:, gk * 128:(gk + 1) * 128],
                        u2_ap(g, c * TCH, (c + 1) * TCH),
                        mix_sb[:, g * 128:(g + 1) * 128],
                        start=True, stop=True,
                    )
                evac(wr[:, gq * 4:(gq + 1) * 4],
                     ps2[:].rearrange("p (g i s) -> p g i s", g=4, i=K))

            def s2(c):
                for gq in range(8):
                    s2_gq(c, gq)

            def t2(c):
                for grp in range(4):
                    pst = pstp.tile([128, 1024], BF16, tag="pst")
                    for k in range(8):
                        ik = grp * 8 + k     # i*2 + kt
                        nc.tensor.transpose(
                            pst[:, k * 128:(k + 1) * 128],
                            w_ts[c][:, ik * 128:(ik + 1) * 128],
                            id_sb[:],
                        )
                    evac(w_ts[c][:, grp * 1024:(grp + 1) * 1024], pst[:])

            def s3(c):
                # y staged in halves (smaller SBUF footprint); out on sync,
                # which is idle during the s2/t2/s3 phase.
                for yh in range(2):
                    y_t = ypool.tile([128, D // 2], BF16, tag="y")
                    for iph in range(4):
                        ip = yh * 4 + iph    # i-pair
                        ps3 = ps3p.tile([128, 512], F32, tag="ps3")
                        for ih in range(2):
                            i = ip * 2 + ih
                            for kt in range(2):
                                nc.tensor.matmul(
                                    ps3[:, ih * 256:(ih + 1) * 256],
                                    w_ts[c][:, (i * 2 + kt) * 128:(i * 2 + kt + 1) * 128],
                                    r_sb[:, kt * 256:(kt + 1) * 256],
                                    start=(kt == 0), stop=(kt == 1),
                                )
                        evac(y_t[:, iph * 512:(iph + 1) * 512], ps3[:])
                    nc.sync.dma_start(
                        yD[c * TCH:(c + 1) * TCH,
                           yh * (D // 2):(yh + 1) * (D // 2)], y_t[:])

            # Phase A: stage-2 gq-outer over the first 6 chunks, so the PE
            # consumes shuffle-1 groups in ring-arrival order instead of
            # stalling on chunk 0's last groups.  Phase B: t2/s3 sweep
            # (no shuffle dependency), with the last 2 chunks' stage-2
            # interleaved once their w buffers free up.
            NPRE = 6
            for gq in range(8):
                for c in range(NPRE):
                    s2_gq(c, gq)
            for it in range(NCH2 + 1):
                if it < NCH2:
                    t2(it)
                if it >= 1:
                    s3(it - 1)
                    if it - 1 + NPRE < NCH2:
                        s2(it - 1 + NPRE)
    nc.compile()
    return nc


# ---------------------------------------------------------------- fft kernel

def _build_fft_nc():
    nc = bacc.Bacc("TRN2", target_bir_lowering=False, debug=False)
    # x_dev: row j*128+p, col tc*1024 + mt*512 + t  (4KB contiguous lines)
    xT = nc.dram_tensor("xT", [K * 128, 2 * NT], BF16, kind="ExternalInput").ap()
    tb_d = nc.dram_tensor("tb", [128, 512], BF16, kind="ExternalInput").ap()
    mix_d = nc.dram_tensor("mix", [128, 32 * 128], BF16, kind="ExternalInput").ap()
    r_d = nc.dram_tensor("rmat", [128, 512], BF16, kind="ExternalInput").ap()
    beta_d = nc.dram_tensor("beta", [128, 32], F32, kind="ExternalInput").ap()
    yT = nc.dram_tensor("yT", [D, NT], BF16, kind="ExternalOutput").ap()

    ec = [0]

    def evac(dst, src):
        # alternate PSUM->SBUF evacuation between DVE and ACT
        if ec[0] % 2 == 0:
            nc.vector.tensor_copy(dst, src)
        else:
            nc.scalar.copy(dst, src)
        ec[0] += 1

    def evac_act(dst, src):
        nc.scalar.copy(dst, src)

    sc = [0]
    shuf_mode = os.environ.get("KERNEL_SHUF", "hw")

    def shuffle_dma(dst, src):
        if shuf_mode == "hw":
            # HWDGE: alternate between the two HWDGE engines (sync, scalar)
            if sc[0] % 2 == 0:
                nc.sync.dma_start(dst, src)
            else:
                nc.scalar.dma_start(dst, src)
        else:
            # SWDGE: sbuf<->sbuf spreads across all 16 engines
            nc.gpsimd.dma_start(dst, src)
        sc[0] += 1

    NTH = NT // TC  # 512-token matmul halves within the full 1024 extent

    with tile.TileContext(nc) as tc:
        with (
            tc.tile_pool(name="consts", bufs=1) as consts,
            tc.tile_pool(name="xpool", bufs=3) as xpool,
            tc.tile_pool(name="upool", bufs=2) as upool,
            tc.tile_pool(name="u2pool", bufs=10) as u2pool,
            tc.tile_pool(name="v2pool", bufs=6) as v2pool,
            tc.tile_pool(name="vpool", bufs=1) as vpool,
            tc.tile_pool(name="ypool", bufs=3) as ypool,
            tc.tile_pool(name="psA", bufs=2, space="PSUM") as psA,
            tc.tile_pool(name="psB", bufs=3, space="PSUM") as psB,
            tc.tile_pool(name="psC", bufs=3, space="PSUM") as psC,
        ):
            tb_sb = consts.tile([128, 512], BF16)
            nc.sync.dma_start(tb_sb[:], tb_d[:])
            mix_sb = consts.tile([128, 32 * 128], BF16)
            nc.sync.dma_start(mix_sb[:], mix_d[:])
            r_sb = consts.tile([128, 512], BF16)
            nc.sync.dma_start(r_sb[:], r_d[:])
            beta_sb = consts.tile([128, 32], F32)
            nc.sync.dma_start(beta_sb[:], beta_d[:])

            # ---- stage 1: per block j, real-DFT projection (fp32r) ----
            u_sb = []
            for pb in range(2):
                u_pb = upool.tile([128, K * NT], BF16, tag="u")
                u_sb.append(u_pb)
            for j in range(K):
                x_t = xpool.tile([128, 2 * NT], BF16, tag="x")
                nc.sync.dma_start(x_t[:], xT[j * 128:(j + 1) * 128, :])
                for pb in range(2):
                    for th in range(NTH):
                        ps1 = psA.tile([128, TC], F32, tag="ps1")
                        for mt in range(2):
                            nc.tensor.matmul(
                                ps1[:],
                                tb_sb[:, (mt * 2 + pb) * 128:(mt * 2 + pb + 1) * 128],
                                x_t[:, mt * NT + th * TC:mt * NT + (th + 1) * TC],
                                start=(mt == 0), stop=(mt == 1),
                            )
                        evac_act(u_sb[pb][:, j * NT + th * TC:j * NT + (th + 1) * TC],
                                 ps1[:])

            # ---- per pb-half: shuffle1 -> stage 2 -> shuffle2 ----
            v_sb = vpool.tile([128, 32 * NT], BF16, tag="v")
            for pb in range(2):
                for gl in range(16):
                    G = 16 * pb + gl
                    u2_t = u2pool.tile([128, NT], BF16, tag="u2")
                    shuffle_dma(u2_t[:], u_sb[pb][8 * gl:8 * gl + 8, :])
                    v2_t = v2pool.tile([128, NT], BF16, tag="v2")
                    for th in range(NTH):
                        ps2 = psB.tile([128, TC], F32, tag="ps2")
                        nc.tensor.matmul(
                            ps2[:],
                            mix_sb[:, G * 128:(G + 1) * 128],
                            u2_t[:, th * TC:(th + 1) * TC],
                            start=True, stop=True,
                        )
                        nc.vector.tensor_scalar_add(
                            v2_t[:, th * TC:(th + 1) * TC], ps2[:],
                            beta_sb[:, G:G + 1])
                    kt = G // 16
                    shuffle_dma(
                        v_sb[8 * gl:8 * gl + 8, kt * 16 * NT:(kt + 1) * 16 * NT],
                        v2_t[:],
                    )

            # ---- stage 3: per output block i, inverse basis + bias ----
            for i in range(K):
                for nb in range(2):
                    ob = i * 2 + nb
                    y_t = ypool.tile([128, NT], BF16, tag="y")
                    for th in range(NTH):
                        ps3 = psC.tile([128, TC], F32, tag="ps3")
                        for kt in range(2):
                            nc.tensor.matmul(
                                ps3[:],
                                r_sb[:, (kt * 2 + nb) * 128:(kt * 2 + nb + 1) * 128],
                                v_sb[:, (kt * 16 + i) * NT + th * TC:
                                     (kt * 16 + i) * NT + (th + 1) * TC],
                                start=(kt == 0), stop=(kt == 1),
                            )
                        evac(y_t[:, th * TC:(th + 1) * TC], ps3[:])
                    nc.scalar.dma_start(yT[ob * 128:(ob + 1) * 128, :], y_t[:])
    nc.compile()
    return nc


# ---------------------------------------------------------------- dense kernel

def _build_dense_nc():
    nc = bacc.Bacc("TRN2", target_bir_lowering=False, debug=False)
    xT = nc.dram_tensor("xT", [D, NT], F32R, kind="ExternalInput").ap()
    m = nc.dram_tensor("m", [D, D], F32R, kind="ExternalInput").ap()
    bias = nc.dram_tensor("bias", [D], F32, kind="ExternalInput").ap()
    yT = nc.dram_tensor("yT", [D, NT], F32, kind="ExternalOutput").ap()

    KT = D // 128
    OB = D // 128
    TH = NT // 512

    with tile.TileContext(nc) as tc:
        with (
            tc.tile_pool(name="xpool", bufs=KT) as xpool,
            tc.tile_pool(name="mpool", bufs=3) as mpool,
            tc.tile_pool(name="bpool", bufs=1) as bpool,
            tc.tile_pool(name="opool", bufs=6) as opool,
            tc.tile_pool(name="psum", bufs=8, space="PSUM") as psum_pool,
        ):
            bias_t = bpool.tile([128, OB], F32)
            nc.sync.dma_start(bias_t[:], bias.rearrange("(c p) -> p c", p=128))

            xts = []
            for kt in range(KT):
                xt = xpool.tile([128, NT], F32R, tag="x")
                nc.sync.dma_start(xt[:], xT[kt * 128:(kt + 1) * 128, :])
                xts.append(xt)

            for ob in range(OB):
                mt = mpool.tile([128, KT * 128], F32R, tag="m")
                nc.sync.dma_start(
                    mt[:].rearrange("p (t o) -> p t o", t=KT),
                    m[:, ob * 128:(ob + 1) * 128].rearrange("(t p) o -> p t o", p=128),
                )
                for th in range(TH):
                    ps = psum_pool.tile([128, 512], F32)
                    for kt in range(KT):
                        nc.tensor.matmul(
                            ps[:],
                            mt[:, kt * 128:(kt + 1) * 128],
                            xts[kt][:, th * 512:(th + 1) * 512],
                            start=(kt == 0), stop=(kt == KT - 1),
                        )
                    osb = opool.tile([128, 512], F32, tag="o")
                    nc.vector.tensor_scalar_add(osb[:], ps[:], bias_t[:, ob:ob + 1])
                    nc.sync.dma_start(
                        yT[ob * 128:(ob + 1) * 128, th * 512:(th + 1) * 512], osb[:])
    nc.compile()
    return nc


# ---------------------------------------------------------------- entry point

def _run(nc, in_maps):
    global LAST_EXEC_NS
    trace = bool(os.environ.get("BASS_TRACE"))
    res = bass_utils.run_bass_kernel_spmd(
        nc, in_maps, list(range(N_CORES)), trace=trace,
        tmpdir=os.environ.get("BASS_TRACE_DIR") or None,
    )
    LAST_EXEC_NS = res.exec_time_ns
    return res


def kernel(x, W, d_bernoulli, bias):
    x = np.asarray(x, dtype=np.float32)
    W = np.asarray(W, dtype=np.float32)
    d_bernoulli = np.asarray(d_bernoulli, dtype=np.float32)
    bias = np.asarray(bias, dtype=np.float32)

    impl = os.environ.get("KERNEL_IMPL", "v2")
    xT = np.ascontiguousarray((x * d_bernoulli[None, :]).T)

    if impl == "v2":
        if "v2" not in _CACHE:
            _CACHE["v2"] = _build_v2_nc()
        t, mix, rd, ident = _v2_host_mats(W)
        xTb = xT.astype(BF16_NP)
        in_maps = []
        for c in range(N_CORES):
            xs = xTb[:, c * NT:(c + 1) * NT]                   # (D, NT)
            xd = (xs.reshape(K, 2, 128, NT)
                  .transpose(0, 2, 1, 3)
                  .reshape(K * 128, 2 * NT))
            in_maps.append({
                "xT": np.ascontiguousarray(xd),
                "tmat": t, "mix": mix, "rmat": rd, "ident": ident,
            })
        res = _run(_CACHE["v2"], in_maps)
        out = np.empty((B, D), dtype=np.float32)
        for c in range(N_CORES):
            out[c * NT:(c + 1) * NT, :] = \
                res.results[c]["yD"].astype(np.float32) + bias[None, :]
        return out

    if impl == "dense":
        if "dense" not in _CACHE:
            _CACHE["dense"] = _build_dense_nc()
        midx = (np.arange(BS)[:, None] - np.arange(BS)[None, :]) % BS
        M = np.empty((D, D), dtype=np.float32)
        for i in range(K):
            for j in range(K):
                M[j * BS:(j + 1) * BS, i * BS:(i + 1) * BS] = W[i, j][midx]
        in_maps = [
            {"xT": np.ascontiguousarray(xT[:, c * NT:(c + 1) * NT]),
             "m": M, "bias": bias}
            for c in range(N_CORES)
        ]
        res = _run(_CACHE["dense"], in_maps)
    else:
        if "fft" not in _CACHE:
            _CACHE["fft"] = _build_fft_nc()
        tb, mix, rd, beta = _fft_host_mats(W, bias)
        in_maps = []
        xTb = xT.astype(BF16_NP)
        for c in range(N_CORES):
            xs = xTb[:, c * NT:(c + 1) * NT]                   # (D, NT)
            # device layout: row j*128+p, col mt*NT + t  (4KB contiguous lines)
            xd = (xs.reshape(K, 2, 128, NT)
                  .transpose(0, 2, 1, 3)
                  .reshape(K * 128, 2 * NT))
            in_maps.append({
                "xT": np.ascontiguousarray(xd),
                "tb": tb, "mix": mix, "rmat": rd, "beta": beta,
            })
        res = _run(_CACHE["fft"], in_maps)

    out = np.empty((B, D), dtype=np.float32)
    for c in range(N_CORES):
        out[c * NT:(c + 1) * NT, :] = res.results[c]["yT"].T.astype(np.float32)
    return out

